# revision 2
# baseline (speedup 1.0000x reference)
"""Trainium2 Bass kernel for streaming dot-product attention with alpha decay.

Math: with e~_s = alpha^{-s} exp(qk_s) the scan becomes a prefix sum computed
as a triangular-ones matmul; QKV_0/Z_0 enter via row-0 fold / K=1 matmul.

v2 layout/engine strategy (vs v1):
- All input transposes (qT, kT, ksT, vst) are done on HOST; device loads them
  directly.  v_init arrives chunked [4, 128, D+1] with a baked ones-column.
- Output is fp16, DRAM layout [BL, T+1, N1, D] (b-major) so each b's stream
  rows go out as one 1 MB contiguous DMA; host transposes/casts back.
- R-build: even b -> GpSimd 1x broadcast TT; odd b -> ACT produces ebexp
  (exp fused with d-expansion, dense fp16) so DVE's TT runs in 2x mode.
- divide: pnum pairs [T,1024] fp32 psum; 3 units on DVE direct (1x),
  1 unit via ACT evac (fp16) + GpSimd dense TT.
"""

import math
from contextlib import ExitStack

import numpy as np

import concourse.bass as bass
import concourse.bacc as bacc
import concourse.tile as tile
from concourse import mybir
from concourse.bass_utils import run_bass_kernel_spmd

ALPHA = 0.99
B, N1, N2, D, T = 64, 64, 512, 64, 128
NCORES = 8
BL = B // NCORES
F32 = mybir.dt.float32
F16 = mybir.dt.float16
Exp = mybir.ActivationFunctionType.Exp
Copy = mybir.ActivationFunctionType.Copy


def _build():
    nc = bacc.Bacc("TRN2", target_bir_lowering=False, debug=False)

    qT_d = nc.dram_tensor("qT", [BL, D, N1], F16, kind="ExternalInput")
    kT_d = nc.dram_tensor("kT", [BL, D, N2], F16, kind="ExternalInput")
    vin_d = nc.dram_tensor("vin", [BL, 4, 128, D + 1], F16, kind="ExternalInput")
    ksT_d = nc.dram_tensor("ksT", [BL, D, T], F16, kind="ExternalInput")
    vst_d = nc.dram_tensor("vst", [BL, T, D], F16, kind="ExternalInput")
    tri_d = nc.dram_tensor("tri", [T, T], F16, kind="ExternalInput")
    sb_d = nc.dram_tensor("sbias", [T, 1], F32, kind="ExternalInput")
    out_d = nc.dram_tensor("out", [BL, T + 1, N1, D], F16, kind="ExternalOutput")

    with tile.TileContext(nc) as tc, ExitStack() as ctx:
        consts = ctx.enter_context(tc.tile_pool(name="consts", bufs=1))
        inbuf = ctx.enter_context(tc.tile_pool(name="inbuf", bufs=1))
        small = ctx.enter_context(tc.tile_pool(name="small", bufs=4))
        ebuf = ctx.enter_context(tc.tile_pool(name="ebuf", bufs=2))
        rbuf = ctx.enter_context(tc.tile_pool(name="rbuf", bufs=2))
        obuf = ctx.enter_context(tc.tile_pool(name="obuf", bufs=2))
        psum = ctx.enter_context(tc.tile_pool(name="psum", bufs=1, space="PSUM"))

        tri = consts.tile([T, T], F16)
        nc.sync.dma_start(out=tri[:], in_=tri_d[:])
        sbias = consts.tile([T, 1], F32)
        nc.sync.dma_start(out=sbias[:], in_=sb_d[:])

        qT_all = inbuf.tile([D, BL, N1], F16)
        kT_all = inbuf.tile([D, BL, N2], F16)
        ksT_all = inbuf.tile([D, BL, T], F16)
        vin_all = inbuf.tile([128, BL, 4, D + 1], F16)
        vst_all = inbuf.tile([T, BL, D], F16)
        o0all = inbuf.tile([N1, BL, D], F16)

        nc.sync.dma_start(out=qT_all[:], in_=qT_d.rearrange("b d n -> d b n"))
        nc.scalar.dma_start(out=kT_all[:], in_=kT_d.rearrange("b d m -> d b m"))
        nc.sync.dma_start(out=ksT_all[:], in_=ksT_d.rearrange("b d t -> d b t"))
        nc.scalar.dma_start(
            out=vin_all[:], in_=vin_d.rearrange("b c p e -> p b c e")
        )
        nc.sync.dma_start(out=vst_all[:], in_=vst_d.rearrange("b t d -> t b d"))

        for b in range(BL):
            qT = qT_all[:, b, :]
            use_gps_r = b % 2 == 0

            # init attention logits: qk[c] [128, 64] = kT_c^T q
            qk_ps = psum.tile([128, 4, N1], F32, tag="pqk", bufs=2)
            for c in range(4):
                nc.tensor.matmul(
                    qk_ps[:, c, :], kT_all[:, b, 128 * c : 128 * (c + 1)], qT,
                    start=True, stop=True,
                )
            qke = small.tile([128, 4, N1], F16, tag="qke")
            nc.scalar.activation(qke[:], qk_ps[:], Exp)

            # [QKV_0 | Z_0]: p0 [64, 65]
            p0 = psum.tile([N1, D + 1], F32, tag="ptr", bufs=2)
            for c in range(4):
                nc.tensor.matmul(
                    p0[:], qke[:, c, :], vin_all[:, b, c, :],
                    start=(c == 0), stop=(c == 3),
                )

            # stream logits ps_s [T, N1]
            ps_s = psum.tile([T, N1], F32, tag="pqk", bufs=2)
            nc.tensor.matmul(ps_s[:], ksT_all[:, b, :], qT, start=True, stop=True)

            # e~: GpSimd-R b's get plain eb [T,N1]; DVE-R b's get dense
            # ebexp [T,N1,D] (exp fused with the d-expansion on ACT)
            if use_gps_r:
                eb = small.tile([T, N1], F16, tag="eb")
                nc.scalar.activation(eb[:], ps_s[:], Exp, bias=sbias[:], scale=1.0)
                eb_mm = eb[:]
            else:
                ebexp = ebuf.tile([T, N1, D], F16, tag="ebexp")
                nc.scalar.activation(
                    ebexp[:],
                    ps_s[:, :, None].broadcast_to([T, N1, D]),
                    Exp, bias=sbias[:], scale=1.0,
                )
                eb_mm = ebexp[:, :, 0:1].rearrange("t n o -> t (n o)")

            # out0 = QKV_0/Z_0 into o0all; fp16 copies for folds
            rz = small.tile([N1, 1], F32, tag="rz")
            nc.vector.reciprocal(rz[:], p0[:, D : D + 1])
            nc.vector.tensor_scalar_mul(o0all[:, b, :], p0[:, 0:D], rz[:])
            qkv0_h = small.tile([N1, D], F16, tag="qkv0h")
            nc.vector.tensor_copy(qkv0_h[:], p0[:, 0:D])
            zcol_h = small.tile([N1, 1], F16, tag="zcolh")
            nc.vector.tensor_copy(zcol_h[:], p0[:, D : D + 1])
            z0f = small.tile([1, N1], F16, tag="z0f")
            nc.sync.dma_start(out=z0f[:], in_=zcol_h[:, :])

            # R[s,n,d] = e~[s,n] * v[s,d]
            R_t = rbuf.tile([T, N1, D], F16, tag="R")
            if use_gps_r:
                nc.gpsimd.tensor_mul(
                    R_t[:],
                    eb[:, :, None].broadcast_to([T, N1, D]),
                    vst_all[:, b, None, :].broadcast_to([T, N1, D]),
                )
            else:
                nc.vector.tensor_mul(
                    R_t[:],
                    ebexp[:],
                    vst_all[:, b, None, :].broadcast_to([T, N1, D]),
                )
            nc.gpsimd.dma_start(
                out=R_t[0:1, :, :], in_=qkv0_h[:, None, :],
                accum_op=mybir.AluOpType.add,
            )

            # den + reciprocal
            pden = psum.tile([T, N1], F32, tag="pqk", bufs=2)
            nc.tensor.matmul(pden[:], tri[:], eb_mm, start=True, stop=False)
            nc.tensor.matmul(pden[:], tri[0:1, :], z0f[:], start=False, stop=True)
            r_t = small.tile([T, N1], F32, tag="r")
            nc.vector.reciprocal(r_t[:], pden[:])
            rh = small.tile([T, N1], F16, tag="rh")
            nc.vector.tensor_copy(rh[:], r_t[:])

            # numerator matmuls in pairs -> [T, 2, 512] psum (2 banks x 2 bufs)
            obig = obuf.tile([T, N1, D], F16, tag="obig")
            for pair in range(4):
                pnum = psum.tile([T, 2, 512], F32, tag="pbig", bufs=2)
                for h in range(2):
                    c = 2 * pair + h
                    nc.tensor.matmul(
                        pnum[:, h, :], tri[:],
                        R_t[:, 8 * c : 8 * (c + 1), :].rearrange(
                            "t n d -> t (n d)"
                        ),
                        start=True, stop=True,
                    )
                ns = slice(16 * pair, 16 * (pair + 1))
                pview = pnum[:].rearrange("t h (n d) -> t (h n) d", d=D)
                if pair < 3:
                    # DVE direct divide (fp32 psum, 1x)
                    nc.vector.tensor_mul(
                        obig[:, ns, :],
                        pview,
                        r_t[:, ns, None].broadcast_to([T, 16, D]),
                    )
                else:
                    # ACT evac to fp16, GpSimd dense TT divide
                    numh = small.tile([T, 16, D], F16, tag="numh")
                    nc.scalar.activation(numh[:], pview, Copy)
                    nc.gpsimd.tensor_mul(
                        obig[:, ns, :],
                        numh[:],
                        rh[:, ns, None].broadcast_to([T, 16, D]),
                    )

            eng = nc.sync if b % 2 == 0 else nc.scalar
            eng.dma_start(
                out=out_d[b, 1:], in_=obig[:].rearrange("t n d -> t (n d)")
            )

        # out0 rows for all b in one DMA
        nc.sync.dma_start(
            out=out_d[:, 0].rearrange("b n d -> n b d"), in_=o0all[:]
        )

    nc.compile()
    return nc


_CACHE = {}


def _get_nc():
    if "nc" not in _CACHE:
        _CACHE["nc"] = _build()
    return _CACHE["nc"]


def _in_maps(q, k_init, v_init, k_stream, v_stream):
    q = np.asarray(q, np.float32).astype(np.float16)
    k_init = np.asarray(k_init, np.float32).astype(np.float16)
    v_init = np.asarray(v_init, np.float32).astype(np.float16)
    k_stream = np.asarray(k_stream, np.float32).astype(np.float16)
    v_stream = np.asarray(v_stream, np.float32).astype(np.float16)

    qT = np.ascontiguousarray(q.transpose(0, 2, 1))            # [B, D, N1]
    kT = np.ascontiguousarray(k_init.transpose(0, 2, 1))       # [B, D, N2]
    vin = np.ones((B, 4, 128, D + 1), np.float16)
    vin[:, :, :, 0:D] = v_init.reshape(B, 4, 128, D)
    ksT = np.ascontiguousarray(k_stream.transpose(1, 2, 0))    # [B, D, T]
    vst = np.ascontiguousarray(v_stream.transpose(1, 0, 2))    # [B, T, D]

    tri = np.triu(np.ones((T, T), np.float32)).astype(np.float16)
    sbias = (np.arange(1, T + 1, dtype=np.float64) * (-math.log(ALPHA))).astype(
        np.float32
    ).reshape(T, 1)
    maps = []
    for i in range(NCORES):
        sl = slice(i * BL, (i + 1) * BL)
        maps.append(
            dict(
                qT=np.ascontiguousarray(qT[sl]),
                kT=np.ascontiguousarray(kT[sl]),
                vin=np.ascontiguousarray(vin[sl]),
                ksT=np.ascontiguousarray(ksT[sl]),
                vst=np.ascontiguousarray(vst[sl]),
                tri=tri,
                sbias=sbias,
            )
        )
    return maps


def run(q, k_init, v_init, attn_mask, k_stream, v_stream, trace=False, **trace_kw):
    """Run on hardware; returns (output, BassKernelResults)."""
    nc = _get_nc()
    maps = _in_maps(q, k_init, v_init, k_stream, v_stream)
    res = run_bass_kernel_spmd(nc, maps, list(range(NCORES)), trace=trace, **trace_kw)
    # per-core out: [BL, T+1, N1, D] fp16 -> full [T+1, B, N1, D] fp32
    out = np.concatenate(
        [res.results[i]["out"].transpose(1, 0, 2, 3) for i in range(NCORES)],
        axis=1,
    ).astype(np.float32)
    return out, res


def kernel(q, k_init, v_init, attn_mask, k_stream, v_stream):
    out, _ = run(q, k_init, v_init, attn_mask, k_stream, v_stream, trace=False)
    return out
